# revision 5
# baseline (speedup 1.0000x reference)
"""Trainium2 Bass kernel for nn_AttenCross (sparse_attention).

reference:
    scores = einsum('bqd,bkd->bqk', Q, D) / sqrt(H)
    scores = where(doc_mask==0, -9999, scores)
    attn   = softmax(scores, -1)
    out    = sum over k of (attn * sim), then sum over q -> (B, 1)

Strategy (8 cores, data-parallel over batch, 2 batches/core):
  - Mask folded into the scores as an additive bias row: a K=1 bf16 matmul
    writes -30000*(1-mask_k) into each PSUM segment before the QK^T matmul
    accumulates on top; exp then underflows to exactly 0 at masked
    columns, so softmax denominator and numerator are both exact.
  - No row-max subtraction: scores ~ N(0,1), exp is safely in range and
    softmax is shift-invariant.
  - PE: Q^T/D^T via transpose-mode matmuls; QK^T in fp32r (full-rate,
    inputs rounded to 12-bit mantissa); per-q softmax normalization is
    folded into a column-sum matmul whose stationary operand is 1/den
    (a [128,32] fp32r tile with only column 0 nonzero); all 64 of those
    column-sum matmuls accumulate into a single [32,512] PSUM bank.
  - ACT: exp(scale*psum) -> SBUF with fused accum_out row-sums (= den).
  - DVE: one fused multiply pass P = E * sim per q-tile; small fixups.
Output per core: [2, 1]; host stacks to [16, 1] fp32.
"""

import numpy as np

import concourse.bacc as bacc
import concourse.tile as tile
import concourse.mybir as mybir
from concourse.bass_utils import run_bass_kernel_spmd
from concourse.masks import make_identity

B, QL, DL, H = 16, 1024, 4096, 128
NCORES = 8
BPC = B // NCORES  # batches per core
QT_N = QL // 128  # 8 q-tiles per batch
KT_N = DL // 128  # 32 k-tiles per batch
SEG = 512
NSEG = DL // SEG  # 8
CH = 1024
NCH = DL // CH  # 4
SCALE = 1.0 / float(np.sqrt(H))

f32 = mybir.dt.float32
f32r = mybir.dt.float32r
bf16 = mybir.dt.bfloat16
i32 = mybir.dt.int32

_CACHED = {}


def _build():
    nc = bacc.Bacc("TRN2", target_bir_lowering=False, debug=False)

    qd = nc.dram_tensor("q", [BPC, QL, H], f32, kind="ExternalInput")
    dd = nc.dram_tensor("d", [BPC, DL, H], f32, kind="ExternalInput")
    sd = nc.dram_tensor("s", [BPC, QL, DL], f32, kind="ExternalInput")
    dmd = nc.dram_tensor("dm", [BPC, DL], i32, kind="ExternalInput")
    outd = nc.dram_tensor("o", [BPC, 1], f32, kind="ExternalOutput")

    with tile.TileContext(nc) as tc:
        with (
            tc.tile_pool(name="const", bufs=1) as const,
            tc.tile_pool(name="bpool", bufs=1) as bpool,
            tc.tile_pool(name="dtp", bufs=2) as dtp,
            tc.tile_pool(name="simp", bufs=2) as simp,
            tc.tile_pool(name="pp", bufs=2) as pp,
            tc.tile_pool(name="ep", bufs=2) as ep,
            tc.tile_pool(name="small", bufs=4) as small,
            tc.tile_pool(name="rp", bufs=2) as rp,
            tc.tile_pool(name="bsm", bufs=2) as bsm,
            tc.tile_pool(name="pscore", bufs=2, space="PSUM") as pscore,
            tc.tile_pool(name="pacc", bufs=1, space="PSUM") as pacc,
            tc.tile_pool(name="ptp", bufs=2, space="PSUM") as ptp,
        ):
            ident = const.tile([128, 128], f32, tag="ident")
            make_identity(nc, ident)
            ones128 = const.tile([128, 1], f32, tag="ones128")
            nc.vector.memset(ones128, 1.0)
            onescol16 = const.tile([1, 128], bf16, tag="onescol16")
            nc.vector.memset(onescol16, 1.0)
            z32 = const.tile([128, 32], f32, tag="z32")
            nc.vector.memset(z32, 0.0)

            for b in range(BPC):
                # ---- per-batch loads ----
                qraw = bpool.tile([128, QT_N, H], f32, tag="qraw")
                nc.sync.dma_start(
                    qraw, qd.ap()[b].rearrange("(t p) h -> p t h", p=128)
                )
                draw = bpool.tile([128, KT_N, H], f32, tag="draw")
                nc.sync.dma_start(
                    draw, dd.ap()[b].rearrange("(t p) h -> p t h", p=128)
                )
                # bias row: -30000*(1-mask) as bf16 on one partition
                mrow = bpool.tile([1, DL], i32, tag="mrow")
                nc.sync.dma_start(mrow, dmd.ap()[b : b + 1, :])
                biasrow = bpool.tile([1, DL], bf16, tag="biasrow")
                nc.vector.tensor_scalar(
                    biasrow, mrow, 30000.0, -30000.0, mybir.AluOpType.mult,
                    mybir.AluOpType.add,
                )

                # ---- transposes: DT[h, k] (fp32r), QT[h, q] (fp32r) ----
                dt = dtp.tile([128, DL], f32r, tag="dt")
                for kt in range(KT_N):
                    tp = ptp.tile([128, 128], f32, tag="tp")
                    nc.tensor.transpose(tp, draw[:, kt, :], ident)
                    nc.vector.tensor_copy(dt[:, kt * 128 : (kt + 1) * 128], tp)
                qt = bpool.tile([128, QL], f32r, tag="qt")
                for t in range(QT_N):
                    tp = ptp.tile([128, 128], f32, tag="tp")
                    nc.tensor.transpose(tp, qraw[:, t, :], ident)
                    nc.vector.tensor_copy(qt[:, t * 128 : (t + 1) * 128], tp)

                # column-sum accumulator: row 0 collects sum_q P[q,k]/den_q
                acc = pacc.tile([32, SEG], f32, tag="acc")

                # ---- q-tiles ----
                for t in range(QT_N):
                    sim_t = simp.tile([128, DL], f32, tag="sim")
                    half = DL // 2
                    nc.sync.dma_start(
                        sim_t[:, :half],
                        sd.ap()[b, t * 128 : (t + 1) * 128, :half],
                    )
                    nc.sync.dma_start(
                        sim_t[:, half:],
                        sd.ap()[b, t * 128 : (t + 1) * 128, half:],
                    )

                    e_t = ep.tile([128, DL], f32, tag="e")
                    den4 = small.tile([128, NCH], f32, tag="den4")
                    for c in range(NCH):
                        psc = pscore.tile([128, CH], f32, tag="sc")
                        for hh in range(CH // SEG):
                            off = c * CH + hh * SEG
                            sl = psc[:, hh * SEG : (hh + 1) * SEG]
                            nc.tensor.matmul(
                                sl,
                                onescol16,
                                biasrow[:, off : off + SEG],
                                start=True,
                                stop=False,
                            )
                            nc.tensor.matmul(
                                sl,
                                qt[:, t * 128 : (t + 1) * 128],
                                dt[:, off : off + SEG],
                                start=False,
                                stop=True,
                            )
                        nc.scalar.activation(
                            out=e_t[:, c * CH : (c + 1) * CH],
                            in_=psc,
                            func=mybir.ActivationFunctionType.Exp,
                            scale=SCALE,
                            accum_out=den4[:, c : c + 1],
                        )

                    den = small.tile([128, 1], f32, tag="den")
                    nc.vector.reduce_sum(den, den4, axis=mybir.AxisListType.X)
                    rv = small.tile([128, 1], f32, tag="rv")
                    nc.vector.reciprocal(rv, den)
                    r32 = rp.tile([128, 32], f32r, tag="r32")
                    nc.vector.tensor_copy(r32, z32)
                    nc.vector.tensor_copy(r32[:, 0:1], rv)

                    p_t = pp.tile([128, DL], f32r, tag="p")
                    nc.vector.tensor_tensor(p_t, e_t, sim_t, mybir.AluOpType.mult)

                    for j in range(NSEG):
                        nc.tensor.matmul(
                            acc,
                            r32,
                            p_t[:, j * SEG : (j + 1) * SEG],
                            start=(t == 0 and j == 0),
                            stop=(t == QT_N - 1 and j == NSEG - 1),
                            skip_group_check=True,
                        )

                # ---- batch epilogue ----
                red32 = bsm.tile([32, 1], f32, tag="red32")
                nc.vector.reduce_sum(red32, acc, axis=mybir.AxisListType.X)
                ps_o = ptp.tile([1, 1], f32, tag="tp")
                nc.tensor.matmul(ps_o, red32, ones128[:32], start=True, stop=True)
                out_sb = bsm.tile([1, 1], f32, tag="out_sb")
                nc.vector.tensor_copy(out_sb, ps_o)
                nc.sync.dma_start(outd.ap()[b : b + 1, :], out_sb)

    nc.compile()
    return nc


def kernel(**inputs: np.ndarray) -> np.ndarray:
    if "nc" not in _CACHED:
        _CACHED["nc"] = _build()
    nc = _CACHED["nc"]

    q = np.ascontiguousarray(np.asarray(inputs["query_input"], dtype=np.float32))
    d = np.ascontiguousarray(np.asarray(inputs["doc_input"], dtype=np.float32))
    s = np.ascontiguousarray(np.asarray(inputs["sim_matrix"], dtype=np.float32))
    dm = np.ascontiguousarray(np.asarray(inputs["doc_mask"], dtype=np.int32))

    in_maps = []
    for c in range(NCORES):
        lo, hi = c * BPC, (c + 1) * BPC
        in_maps.append(
            {
                "q": q[lo:hi],
                "d": d[lo:hi],
                "s": s[lo:hi],
                "dm": dm[lo:hi],
            }
        )

    res = run_bass_kernel_spmd(nc, in_maps, core_ids=list(range(NCORES)))
    out = np.concatenate([res.results[c]["o"] for c in range(NCORES)], axis=0)
    return out.astype(np.float32)
